# revision 36
# baseline (speedup 1.0000x reference)
"""Trainium2 Bass kernel for the coco_DAA loss (nn_DAA_66812511256800).

Math (M = N*K = 320, a = input1.reshape(M, D)):
    score = a @ a.T                                   (M, M), symmetric
    rank_X[b, c] = sum_a mask[a, c] * sig(100*(X[a, b] - X[b, c])) + 1
    out = 1 - mean(min(rank_s, rank_c) / max(rank_s, rank_c))

Key reductions:
  * score is symmetric -> the masked diag term is exactly sig(0) = 0.5:
      score_rank = colsum + 0.5,  colsum[b,c] = sum_a sig(100*(score[b,a]-score[b,c]))
  * cider ranks only depend on block indices (cider_map repeated K times), so the
    M^3 cider reduction collapses to an N^3 = 64^3 one.

Device strategy (8 cores SPMD, reduction axes sharded):
  * PE: score row-tiles via aT-chunk matmuls; the moving operand is extended with
    the per-core 40 aT columns so the per-core bias block B100 = 100*score[:,a_rng]
    falls out of the same matmuls. The 64-row tail tile is computed twice into
    both partition halves so every later instruction runs 128 partitions wide.
  * ScalarE mul x100 -> S100X tiles in SBUF (score || bias block).
  * VectorE: z[b, c] = S100X[b,c] - S100X[b, 320+j]  (tensor_scalar, per a)
  * ScalarE: sigmoid over G*320-wide groups (amortizes the per-instr bubble),
    fp16 output.
  * PE: identity-matmul accumulation of sig tiles over a into PSUM (f32).
  * cider: same bias trick at (64,64): 8 sigmoids + 8 identity matmuls.
Host: sums partials over cores, applies closed-form diag/+1 terms, expands cider
ranks, reduces to the scalar. All O(M^2) numpy glue.
"""

import numpy as np
from contextlib import ExitStack

import concourse.bass as bass
import concourse.bacc as bacc
import concourse.tile as tile
from concourse import mybir
from concourse.bass_utils import run_bass_kernel_spmd
from concourse.masks import make_identity

F32 = mybir.dt.float32
F32R = mybir.dt.float32r
F16 = mybir.dt.float16
AF = mybir.ActivationFunctionType

N_, K_, D_ = 64, 5, 512
M_ = N_ * K_            # 320
NCORES = 8
APC = M_ // NCORES      # 40 a-values per core
IPC = N_ // NCORES      # 8 cider rows per core
DT = 4                  # contraction chunks of 128 over D=512
MX = M_ + APC           # 360: score row || bias block
G = 20                  # sigmoid group size (a-values per ACT instruction)

_CACHE = {}
LAST_RESULTS = None


def _dedup_ldweights(nc):
    """bacc emits one InstLdweights per matmul even when the stationary
    operand (our fp16 identity) never changes; the reload forces every
    reduction matmul to take its full isolated latency (~299 ns) instead
    of pipelining (~136 ns). Matmults here are non-self-loading
    (ldweights=False), so consecutive loads of an identical weights AP can
    be dropped. Loads that carry sync waits/updates are kept."""
    removed = 0
    for fn in nc.m.functions:
        for bb in fn.blocks:
            last_key = None
            keep = []
            for inst in bb.instructions:
                tn = type(inst).__name__
                if tn == "InstLdweights":
                    key = (
                        str(inst.ins[0]),
                        str(getattr(inst, "tile_position", None)),
                        str(getattr(inst, "perf_mode", None)),
                        str(getattr(inst, "is_transpose", None)),
                    )
                    si = inst.sync_info
                    has_sync = bool(si and (si.on_wait or si.on_update))
                    if key == last_key and not has_sync:
                        removed += 1
                        continue
                    last_key = key
                keep.append(inst)
            if removed:
                bb.instructions = keep
    return removed


def _build_program():
    nc = bacc.Bacc(None, target_bir_lowering=False, debug=False)
    # atx[p, d, :] = [aT d-chunk row p (320) | per-core aT a-slice columns (40)]
    atx_d = nc.dram_tensor("atx", [128, DT, MX], F32, kind="ExternalInput").ap()
    # cmx = [cider_map (64, 64) | 100*cm.T per-core i-columns (64, 8)]
    cmx_d = nc.dram_tensor("cmx", [N_, N_ + IPC], F32, kind="ExternalInput").ap()
    # [p, t, ab, c]: three 128-row accumulator tiles x two PSUM banks each
    # (tile2 additionally holds two half-sums across its partition halves)
    colsum_d = nc.dram_tensor(
        "colsum", [128, 3, 2, M_], F32, kind="ExternalOutput"
    ).ap()
    cider_d = nc.dram_tensor("cider", [N_, 2, N_], F32, kind="ExternalOutput").ap()

    with tile.TileContext(nc) as tc, ExitStack() as ctx:
        consts = ctx.enter_context(tc.tile_pool(name="consts", bufs=1))
        zpool = ctx.enter_context(tc.tile_pool(name="zpool", bufs=3))
        sigpool = ctx.enter_context(tc.tile_pool(name="sigpool", bufs=3))
        outp = ctx.enter_context(tc.tile_pool(name="outp", bufs=1))
        ps_w = ctx.enter_context(tc.tile_pool(name="ps_w", bufs=2, space="PSUM"))
        ps_ac = ctx.enter_context(tc.tile_pool(name="ps_ac", bufs=1, space="PSUM"))

        atall = consts.tile([128, DT, MX], F32, tag="atall")
        nc.sync.dma_start(out=atall, in_=atx_d)
        at = [atall[:, d, :] for d in range(DT)]
        cmx = consts.tile([N_, N_ + IPC], F32, tag="cmx")
        nc.sync.dma_start(out=cmx, in_=cmx_d)
        cm = cmx[:, :N_]
        cmt = cmx[:, N_ : N_ + IPC]
        ident = consts.tile([128, 128], F16, tag="ident")
        make_identity(nc, ident)

        # S100X tiles: 100*(score row-tile || per-core bias block), fp16 in SBUF
        # (bias block separately in f32 for tensor_scalar).
        # Tail tile (64 rows) is materialized in both partition halves.
        s100x = []
        b100 = []
        for ti in range(3):
            sp = ps_w.tile([128, MX], F32, tag="scoreps")
            if ti < 2:
                b0 = 128 * ti
                for d in range(DT):
                    nc.tensor.matmul(
                        sp, at[d][:, b0 : b0 + 128], at[d][:, :],
                        start=(d == 0), stop=(d == DT - 1),
                    )
            else:
                for half in range(2):
                    for d in range(DT):
                        nc.tensor.matmul(
                            sp[64 * half : 64 * half + 64, :],
                            at[d][:, 256:320], at[d][:, :],
                            start=(d == 0), stop=(d == DT - 1),
                        )
            sx = consts.tile([128, M_], F16, tag=f"s100x{ti}")
            nc.scalar.mul(sx, sp[:, :M_], 100.0)
            bc = consts.tile([128, APC], F32, tag=f"b100_{ti}")
            nc.scalar.mul(bc, sp[:, M_:MX], 100.0)
            s100x.append(sx)
            b100.append(bc)

        # bias columns for the duplicated tail tile: top half uses j 0..19,
        # bottom half uses j 20..39 (SBUF->SBUF DMA moves across partitions)
        bdup = consts.tile([128, APC // 2], F32, tag="bdup")
        nc.sync.dma_start(out=bdup[0:64, :], in_=b100[2][0:64, 0:20])
        nc.sync.dma_start(out=bdup[64:128, :], in_=b100[2][64:128, 20:40])

        # main loop: z on DVE, grouped sigmoid on ACT, accumulate on PE.
        # Accumulation alternates between two PSUM banks so consecutive
        # matmuls overlap (same-bank accumulate serializes fill/drain).
        njs = [APC, APC, APC // 2]
        outsb = outp.tile([128, 3, 2, M_], F32, tag="outs")
        for ti in range(3):
            acA = ps_ac.tile([128, M_], F32, tag="accA")
            acB = ps_ac.tile([128, M_], F32, tag="accB")
            nj = njs[ti]
            for g0 in range(0, nj, G):
                gn = min(G, nj - g0)
                zb = zpool.tile([128, G * M_], F16, tag="z")
                for k in range(gn):
                    j = g0 + k
                    col = (
                        b100[ti][:, j : j + 1] if ti < 2 else bdup[:, j : j + 1]
                    )
                    nc.vector.tensor_scalar_sub(
                        zb[:, k * M_ : (k + 1) * M_], s100x[ti], col
                    )
                sg = sigpool.tile([128, G * M_], F16, tag="sig")
                nc.scalar.activation(
                    sg[:, : gn * M_], zb[:, : gn * M_], AF.Sigmoid, scale=-1.0
                )
                for k in range(gn):
                    j = g0 + k
                    nc.tensor.matmul(
                        acA if j % 2 == 0 else acB,
                        ident, sg[:, k * M_ : (k + 1) * M_],
                        start=(j < 2), stop=(j >= nj - 2),
                    )
            nc.vector.tensor_copy(outsb[:, ti, 0, :], acA)
            nc.vector.tensor_copy(outsb[:, ti, 1, :], acB)
        nc.sync.dma_start(out=colsum_d, in_=outsb)

        # cider partial: sum_i sig(100*(cm[i, p] - cm[p, q])) over the core's i's
        cmf16 = consts.tile([N_, N_], F16, tag="cmf16")
        nc.scalar.mul(cmf16, cm, 100.0)
        zc = zpool.tile([64, IPC * N_], F16, tag="zc")
        for j in range(IPC):
            nc.vector.tensor_scalar_sub(
                zc[:, j * N_ : (j + 1) * N_], cmf16, cmt[:, j : j + 1]
            )
        sgc = sigpool.tile([64, IPC * N_], F16, tag="sigc")
        nc.scalar.activation(sgc, zc, AF.Sigmoid, scale=-1.0)
        cacA = ps_ac.tile([64, N_], F32, tag="accA")
        cacB = ps_ac.tile([64, N_], F32, tag="accB")
        for j in range(IPC):
            nc.tensor.matmul(
                cacA if j % 2 == 0 else cacB,
                ident[:64, :64], sgc[:, j * N_ : (j + 1) * N_],
                start=(j < 2), stop=(j >= IPC - 2),
            )
        cob = outp.tile([64, 2, N_], F32, tag="outc")
        nc.vector.tensor_copy(cob[:, 0, :], cacA)
        nc.vector.tensor_copy(cob[:, 1, :], cacB)
        nc.sync.dma_start(out=cider_d, in_=cob)

    nc.compile()
    _dedup_ldweights(nc)
    return nc


def _tsig64(x):
    # faithful f64 version of the reference's clipped temperature sigmoid
    e = np.clip(-x / 0.01, -50.0, 50.0)
    return 1.0 / (1.0 + np.exp(e))


def kernel(input1, input2, cider_map):
    global LAST_RESULTS
    if "nc" not in _CACHE:
        _CACHE["nc"] = _build_program()
    nc = _CACHE["nc"]

    a = np.ascontiguousarray(np.asarray(input1, dtype=np.float32).reshape(M_, D_))
    at4 = np.ascontiguousarray(a.T).reshape(DT, 128, M_)
    cm = np.ascontiguousarray(np.asarray(cider_map, dtype=np.float32))
    cmt100 = np.ascontiguousarray(100.0 * cm.T.astype(np.float32))

    in_maps = []
    for c in range(NCORES):
        atx = np.concatenate(
            [at4, at4[:, :, c * APC : (c + 1) * APC]], axis=2
        ).astype(np.float32)
        atx = atx.transpose(1, 0, 2)  # -> [p, d, mx]
        cmx = np.concatenate(
            [cm, cmt100[:, c * IPC : (c + 1) * IPC]], axis=1
        ).astype(np.float32)
        in_maps.append(
            {"atx": np.ascontiguousarray(atx), "cmx": np.ascontiguousarray(cmx)}
        )

    LAST_RESULTS = run_bass_kernel_spmd(nc, in_maps, core_ids=list(range(NCORES)))
    res = LAST_RESULTS.results

    colsum = np.zeros((M_, M_), dtype=np.float64)
    s_cm = np.zeros((N_, N_), dtype=np.float64)
    for r in res:
        cs = r["colsum"].astype(np.float64).sum(axis=2)  # [128, 3, 320]
        colsum[:128] += cs[:, 0, :]
        colsum[128:256] += cs[:, 1, :]
        colsum[256:] += cs[:64, 2, :] + cs[64:, 2, :]
        s_cm += r["cider"].astype(np.float64).sum(axis=1)

    score_rank = colsum + 0.5  # -sig(0) + 1
    cmf = cm.astype(np.float64)
    t2 = _tsig64(cmf.T - cmf)  # [p, q] -> tsig(cm[q, p] - cm[p, q])
    cider_rank_n = K_ * s_cm - t2 + 1.0
    cider_rank = np.repeat(np.repeat(cider_rank_n, K_, axis=0), K_, axis=1)

    mn = np.minimum(cider_rank, score_rank)
    mx = np.maximum(cider_rank, score_rank)
    asp = (mn / mx).mean()
    return np.float32(1.0 - asp)
